# revision 6
# baseline (speedup 1.0000x reference)
"""HeatmapMSELoss Trainium2 kernel.

Computes mean((heatmaps_pred - heatmaps_gt)^2) where heatmaps_gt is an
isotropic 2D gaussian (sigma=1, peak 1) rendered at the projection of each
3D joint into each view.

Key identity: the gaussian separates, gt[h,w] = gy[h] * gx[w], so

  sum_hw (pred - gt)^2 = sum_hw pred^2 - 2 * gy^T (pred @ gx) + (sum gy^2)(sum gx^2)

The 142MB gt tensor is never materialized. Per (b,v,j) slice the device
computes sum(pred^2) (scalar-engine square + accumulate) and
m' = pred^T @ gy (one matmul, PSUM column), then a fused DVE
multiply+reduce against gx. The tiny 1D gaussians (2.2MB total) and the
final scalar combine are done on host in float64.

Sharding: data-parallel over batch, 4 batches per core across 8 cores.
"""

import numpy as np

import concourse.bacc as bacc
import concourse.bass as bass
import concourse.tile as tile
from concourse import mybir
from concourse.bass_utils import run_bass_kernel_spmd

B, V, J, H, W = 32, 4, 17, 128, 128
N_CORES = 8
B_LOC = B // N_CORES          # 4 batches per core
GROUPS = B_LOC * V            # 16 (b,v) groups per core
SLICES = GROUPS * J           # 272 slices per core

_CACHE = {}


def _build_nc():
    # Bacc (not raw Bass): its finalize() runs the legalization passes that
    # split multi-wait instructions (matmul can carry at most 1 sync wait).
    nc = bacc.Bacc()
    f32 = mybir.dt.float32

    pred = nc.declare_dram_parameter("pred", [B_LOC, V, J, H, W], f32, isOutput=False)
    gyt = nc.declare_dram_parameter("gyt", [H, SLICES], f32, isOutput=False)
    gxt = nc.declare_dram_parameter("gxt", [W, SLICES], f32, isOutput=False)
    partials = nc.declare_dram_parameter("partials", [2, 128, GROUPS], f32, isOutput=True)

    with tile.TileContext(nc) as tc:
        with (
            tc.tile_pool(name="consts", bufs=1) as consts,
            tc.tile_pool(name="loads", bufs=3) as loads,
            tc.tile_pool(name="sq", bufs=2) as sqpool,
            tc.tile_pool(name="prod", bufs=2) as prodpool,
            tc.tile_pool(name="psum", bufs=4, space="PSUM") as psumpool,
            tc.tile_pool(name="outs", bufs=1) as outs,
        ):
            gyt_t = consts.tile([H, SLICES], f32)
            nc.sync.dma_start(out=gyt_t[:], in_=gyt[:, :])
            gxt_t = consts.tile([W, SLICES], f32)
            nc.sync.dma_start(out=gxt_t[:], in_=gxt[:, :])

            s1cols = outs.tile([128, GROUPS], f32)
            s2cols = outs.tile([128, GROUPS], f32)

            for g in range(GROUPS):
                b, v = divmod(g, V)
                # [J,H,W] dram block -> SBUF [H, J, W] (partition = h)
                t = loads.tile([H, J, W], f32)
                nc.sync.dma_start(
                    out=t[:], in_=pred[b, v].rearrange("j h w -> h j w")
                )

                # s1: per-partition sum of pred^2 over (j, w)
                sq = sqpool.tile([H, J, W], f32)
                nc.scalar.activation(
                    out=sq[:],
                    in_=t[:],
                    func=mybir.ActivationFunctionType.Square,
                    accum_out=s1cols[:, g : g + 1],
                )

                # s2: m'_j = pred_j^T @ gy_j for each joint -> psum column j
                ps = psumpool.tile([128, J], f32)
                for j in range(J):
                    s = g * J + j
                    nc.tensor.matmul(
                        ps[:, j : j + 1],
                        t[:, j, :],
                        gyt_t[:, s : s + 1],
                        start=True,
                        stop=True,
                    )
                # dot each column with gx, then per-partition sum over joints
                prod = prodpool.tile([128, J], f32)
                nc.vector.tensor_mul(
                    prod[:], ps[:], gxt_t[:, g * J : (g + 1) * J]
                )
                nc.vector.reduce_sum(
                    s2cols[:, g : g + 1], prod[:], axis=mybir.AxisListType.X
                )

            nc.sync.dma_start(out=partials[0], in_=s1cols[:])
            nc.sync.dma_start(out=partials[1], in_=s2cols[:])

    nc.finalize()  # Bacc: runs legalization (wait splitting) + regalloc
    return nc


def _gaussians(proj_mats_batch, joints_3d_gt_batch):
    """1D gaussians gy [B,V,J,H], gx [B,V,J,W] in float32 (reference math)."""
    joints = joints_3d_gt_batch.astype(np.float32)
    ones = np.ones(joints.shape[:-1] + (1,), dtype=np.float32)
    joints_h = np.concatenate([joints, ones], axis=-1)  # [B, J, 4]
    proj = np.einsum(
        "bvcd,bjd->bvjc", proj_mats_batch.astype(np.float32), joints_h
    ).astype(np.float32)  # [B, V, J, 3]
    joints_2d = proj[..., :2] / proj[..., 2:3]  # (x, y)
    xs = np.arange(W, dtype=np.float32)
    ys = np.arange(H, dtype=np.float32)
    dx2 = (xs - joints_2d[..., 0, None]) ** 2  # [B,V,J,W]
    dy2 = (ys - joints_2d[..., 1, None]) ** 2  # [B,V,J,H]
    gx = np.exp(-0.5 * dx2).astype(np.float32)
    gy = np.exp(-0.5 * dy2).astype(np.float32)
    return gy, gx


def kernel(heatmaps_pred, proj_mats_batch, joints_3d_gt_batch, joints_3d_valid_batch,
           _profile=None):
    heatmaps_pred = np.ascontiguousarray(np.asarray(heatmaps_pred, dtype=np.float32))
    gy, gx = _gaussians(np.asarray(proj_mats_batch), np.asarray(joints_3d_gt_batch))

    # s3 = sum over slices of (sum_h gy^2) * (sum_w gx^2), exact in f64
    s3 = float(
        ((gy.astype(np.float64) ** 2).sum(-1) * (gx.astype(np.float64) ** 2).sum(-1)).sum()
    )

    if "nc" not in _CACHE:
        _CACHE["nc"] = _build_nc()
    nc = _CACHE["nc"]

    in_maps = []
    for c in range(N_CORES):
        bsl = slice(B_LOC * c, B_LOC * (c + 1))
        # slice order: (b_local, v, j) -> s ; tiles are [H|W, SLICES]
        gyt = np.ascontiguousarray(gy[bsl].reshape(SLICES, H).T)
        gxt = np.ascontiguousarray(gx[bsl].reshape(SLICES, W).T)
        in_maps.append(
            {
                "pred": heatmaps_pred[bsl],
                "gyt": gyt,
                "gxt": gxt,
            }
        )

    res = run_bass_kernel_spmd(nc, in_maps, core_ids=list(range(N_CORES)))
    if _profile is not None:
        _profile["result"] = res
        _profile["in_maps"] = in_maps

    s1 = 0.0
    s2 = 0.0
    for c in range(N_CORES):
        p = res.results[c]["partials"].astype(np.float64)
        s1 += p[0].sum()
        s2 += p[1].sum()

    total = s1 - 2.0 * s2 + s3
    return np.float32(total / (B * V * J * H * W))


# revision 11
# speedup vs baseline: 2.2921x; 2.2921x over previous
"""HeatmapMSELoss Trainium2 kernel.

Computes mean((heatmaps_pred - heatmaps_gt)^2) where heatmaps_gt is an
isotropic 2D gaussian (sigma=1, peak 1) rendered at the projection of each
3D joint into each view.

Key identity: the gaussian separates, gt[h,w] = gy[h] * gx[w], so

  sum_hw (pred - gt)^2 = sum_hw pred^2 - 2 * gy^T (pred @ gx) + (sum gy^2)(sum gx^2)

The 142MB gt tensor is never materialized. Per (b,v,j) slice the device
computes sum(pred^2) (scalar-engine square + accumulate) and
m' = pred^T @ gy (one matmul, PSUM column), then a fused DVE
multiply+reduce against gx. The tiny 1D gaussians (2.2MB total) and the
final scalar combine are done on host in float64.

Sharding: data-parallel over batch, 4 batches per core across 8 cores.
"""

import numpy as np

import concourse.bacc as bacc
import concourse.bass as bass
import concourse.tile as tile
from concourse import mybir
from concourse.bass_utils import run_bass_kernel_spmd

B, V, J, H, W = 32, 4, 17, 128, 128
N_CORES = 8
B_LOC = B // N_CORES          # 4 batches per core
GROUPS = B_LOC * V            # 16 (b,v) groups per core
SLICES = GROUPS * J           # 272 slices per core

_CACHE = {}


GPB = 2                    # (b,v) groups per block
NBLK = GROUPS // GPB       # blocks per core
JB = GPB * J               # joints (slices) per block

# chunk sizes (in slices) over the 272 per-core slices: small chunks at the
# start (fast pipeline ramp: compute starts after a ~1us DMA, not ~3us) and
# at the end (short tail after the last DMA lands)
CHUNKS = [4, 4, 4, 5] + [17] * 14 + [9, 8]
assert sum(CHUNKS) == SLICES


def _build_nc(passes=1, chunks=None, load_bufs=4):
    # Bacc (not raw Bass): its finalize() runs the legalization passes that
    # split multi-wait instructions (matmul can carry at most 1 sync wait).
    nc = bacc.Bacc()
    f32 = mybir.dt.float32
    chunks = list(CHUNKS) if chunks is None else list(chunks)
    nck = len(chunks)
    maxck = max(chunks)

    pred = nc.declare_dram_parameter("pred", [SLICES, H, W], f32, isOutput=False)
    gyt = nc.declare_dram_parameter("gyt", [H, SLICES], f32, isOutput=False)
    gxt = nc.declare_dram_parameter("gxt", [W, SLICES], f32, isOutput=False)
    partials = nc.declare_dram_parameter("partials", [128, 2, nck], f32, isOutput=True)

    with tile.TileContext(nc) as tc:
        with (
            tc.tile_pool(name="consts", bufs=1) as consts,
            tc.tile_pool(name="loads", bufs=load_bufs) as loads,
            tc.tile_pool(name="sq", bufs=2) as sqpool,
            tc.tile_pool(name="prod", bufs=2) as prodpool,
            tc.tile_pool(name="psum", bufs=4, space="PSUM") as psumpool,
            tc.tile_pool(name="outs", bufs=1) as outs,
        ):
            # warm-up ACT so the Square table-set load (~2.7us) overlaps the
            # first pred DMA instead of stalling the first real ACT
            warm = consts.tile([128, 1], f32)
            nc.vector.memset(warm[:], 0.0)
            wsq = consts.tile([128, 1], f32)
            nc.scalar.activation(
                out=wsq[:], in_=warm[:], func=mybir.ActivationFunctionType.Square
            )

            gyt_t = consts.tile([H, SLICES], f32)
            nc.sync.dma_start(out=gyt_t[:], in_=gyt[:, :])
            gxt_t = consts.tile([W, SLICES], f32)
            nc.sync.dma_start(out=gxt_t[:], in_=gxt[:, :])

            outcols = outs.tile([128, 2, nck], f32)

            for _p in range(passes):
                s0 = 0
                for c, csz in enumerate(chunks):
                    t = loads.tile([H, maxck, W], f32, tag="loads")
                    nc.sync.dma_start(
                        out=t[:, :csz, :],
                        in_=pred[s0 : s0 + csz].rearrange("s h w -> h s w"),
                    )

                    # s1: per-partition sum of pred^2 over (s, w)
                    sq = sqpool.tile([H, maxck, W], f32, tag="sq")
                    nc.scalar.activation(
                        out=sq[:, :csz, :],
                        in_=t[:, :csz, :],
                        func=mybir.ActivationFunctionType.Square,
                        accum_out=outcols[:, 0, c : c + 1],
                    )

                    # s2: m'_s = pred_s^T @ gy_s per slice -> psum column
                    ps = psumpool.tile([128, maxck], f32, tag="psum")
                    for sj in range(csz):
                        s = s0 + sj
                        nc.tensor.matmul(
                            ps[:, sj : sj + 1],
                            t[:, sj, :],
                            gyt_t[:, s : s + 1],
                            start=True,
                            stop=True,
                        )
                    # dot with gx, then per-partition sum over slices
                    prod = prodpool.tile([128, maxck], f32, tag="prod")
                    nc.vector.tensor_mul(
                        prod[:, :csz], ps[:, :csz], gxt_t[:, s0 : s0 + csz]
                    )
                    nc.vector.reduce_sum(
                        outcols[:, 1, c : c + 1], prod[:, :csz],
                        axis=mybir.AxisListType.X,
                    )
                    s0 += csz

            nc.sync.dma_start(out=partials[:, :, :], in_=outcols[:])

    nc.finalize()  # Bacc: runs legalization (wait splitting) + regalloc
    return nc


def _build_nc_gpb(passes=1, gpb=GPB, load_bufs=3):
    # Bacc (not raw Bass): its finalize() runs the legalization passes that
    # split multi-wait instructions (matmul can carry at most 1 sync wait).
    nc = bacc.Bacc()
    f32 = mybir.dt.float32
    nblk = GROUPS // gpb
    jb = gpb * J

    pred = nc.declare_dram_parameter("pred", [B_LOC * V, J, H, W], f32, isOutput=False)
    gyt = nc.declare_dram_parameter("gyt", [H, SLICES], f32, isOutput=False)
    gxt = nc.declare_dram_parameter("gxt", [W, SLICES], f32, isOutput=False)
    partials = nc.declare_dram_parameter("partials", [2, 128, nblk], f32, isOutput=True)

    with tile.TileContext(nc) as tc:
        with (
            tc.tile_pool(name="consts", bufs=1) as consts,
            tc.tile_pool(name="loads", bufs=load_bufs) as loads,
            tc.tile_pool(name="sq", bufs=2) as sqpool,
            tc.tile_pool(name="prod", bufs=2) as prodpool,
            tc.tile_pool(name="psum", bufs=4, space="PSUM") as psumpool,
            tc.tile_pool(name="outs", bufs=1) as outs,
        ):
            gyt_t = consts.tile([H, SLICES], f32)
            nc.sync.dma_start(out=gyt_t[:], in_=gyt[:, :])
            gxt_t = consts.tile([W, SLICES], f32)
            nc.sync.dma_start(out=gxt_t[:], in_=gxt[:, :])

            s1cols = outs.tile([128, nblk], f32)
            s2cols = outs.tile([128, nblk], f32)

            for _p in range(passes):
                for blk in range(nblk):
                    g0 = blk * gpb
                    # gpb contiguous [J,H,W] dram blocks -> SBUF [H, gpb*J, W]
                    t = loads.tile([H, jb, W], f32)
                    nc.sync.dma_start(
                        out=t[:],
                        in_=pred[g0 : g0 + gpb].rearrange("g j h w -> h (g j) w"),
                    )

                    # s1: per-partition sum of pred^2 over (g, j, w)
                    sq = sqpool.tile([H, jb, W], f32)
                    nc.scalar.activation(
                        out=sq[:],
                        in_=t[:],
                        func=mybir.ActivationFunctionType.Square,
                        accum_out=s1cols[:, blk : blk + 1],
                    )

                    # s2: m'_s = pred_s^T @ gy_s per slice -> psum column
                    ps = psumpool.tile([128, jb], f32)
                    for sj in range(jb):
                        s = g0 * J + sj
                        nc.tensor.matmul(
                            ps[:, sj : sj + 1],
                            t[:, sj, :],
                            gyt_t[:, s : s + 1],
                            start=True,
                            stop=True,
                        )
                    # dot with gx, then per-partition sum over slices
                    prod = prodpool.tile([128, jb], f32)
                    nc.vector.tensor_mul(
                        prod[:], ps[:], gxt_t[:, g0 * J : g0 * J + jb]
                    )
                    nc.vector.reduce_sum(
                        s2cols[:, blk : blk + 1], prod[:], axis=mybir.AxisListType.X
                    )

            nc.sync.dma_start(out=partials[0], in_=s1cols[:])
            nc.sync.dma_start(out=partials[1], in_=s2cols[:])

    nc.finalize()  # Bacc: runs legalization (wait splitting) + regalloc
    return nc


def _gaussians(proj_mats_batch, joints_3d_gt_batch):
    """1D gaussians gy [B,V,J,H], gx [B,V,J,W] in float32 (reference math)."""
    joints = joints_3d_gt_batch.astype(np.float32)
    ones = np.ones(joints.shape[:-1] + (1,), dtype=np.float32)
    joints_h = np.concatenate([joints, ones], axis=-1)  # [B, J, 4]
    proj = np.einsum(
        "bvcd,bjd->bvjc", proj_mats_batch.astype(np.float32), joints_h
    ).astype(np.float32)  # [B, V, J, 3]
    joints_2d = proj[..., :2] / proj[..., 2:3]  # (x, y)
    xs = np.arange(W, dtype=np.float32)
    ys = np.arange(H, dtype=np.float32)
    dx2 = (xs - joints_2d[..., 0, None]) ** 2  # [B,V,J,W]
    dy2 = (ys - joints_2d[..., 1, None]) ** 2  # [B,V,J,H]
    gx = np.exp(-0.5 * dx2).astype(np.float32)
    gy = np.exp(-0.5 * dy2).astype(np.float32)
    return gy, gx


def kernel(heatmaps_pred, proj_mats_batch, joints_3d_gt_batch, joints_3d_valid_batch,
           _profile=None):
    heatmaps_pred = np.ascontiguousarray(np.asarray(heatmaps_pred, dtype=np.float32))
    gy, gx = _gaussians(np.asarray(proj_mats_batch), np.asarray(joints_3d_gt_batch))

    # s3 = sum over slices of (sum_h gy^2) * (sum_w gx^2), exact in f64
    s3 = float(
        ((gy.astype(np.float64) ** 2).sum(-1) * (gx.astype(np.float64) ** 2).sum(-1)).sum()
    )

    if "nc" not in _CACHE:
        _CACHE["nc"] = _build_nc()
    nc = _CACHE["nc"]

    in_maps = []
    for c in range(N_CORES):
        bsl = slice(B_LOC * c, B_LOC * (c + 1))
        # slice order: (b_local, v, j) -> s ; tiles are [H|W, SLICES]
        gyt = np.ascontiguousarray(gy[bsl].reshape(SLICES, H).T)
        gxt = np.ascontiguousarray(gx[bsl].reshape(SLICES, W).T)
        in_maps.append(
            {
                "pred": heatmaps_pred[bsl].reshape(SLICES, H, W),
                "gyt": gyt,
                "gxt": gxt,
            }
        )

    res = run_bass_kernel_spmd(nc, in_maps, core_ids=list(range(N_CORES)))
    if _profile is not None:
        _profile["result"] = res
        _profile["in_maps"] = in_maps

    s1 = 0.0
    s2 = 0.0
    for c in range(N_CORES):
        p = res.results[c]["partials"].astype(np.float64)
        s1 += p[:, 0, :].sum()
        s2 += p[:, 1, :].sum()

    total = s1 - 2.0 * s2 + s3
    return np.float32(total / (B * V * J * H * W))


# revision 13
# speedup vs baseline: 5.1380x; 2.2416x over previous
"""HeatmapMSELoss Trainium2 kernel.

Computes mean((heatmaps_pred - heatmaps_gt)^2) where heatmaps_gt is an
isotropic 2D gaussian (sigma=1, peak 1) rendered at the projection of each
3D joint into each view.

Key identity: the gaussian separates, gt[h,w] = gy[h] * gx[w], so

  sum_hw (pred - gt)^2 = sum_hw pred^2 - 2 * gy^T (pred @ gx) + (sum gy^2)(sum gx^2)

The 142MB gt tensor is never materialized. Per (b,v,j) slice the device
computes sum(pred^2) (scalar-engine square + accumulate) and
m' = pred^T @ gy (one matmul, PSUM column), then a fused DVE
multiply+reduce against gx. The tiny 1D gaussians (2.2MB total) and the
final scalar combine are done on host in float64.

Sharding: data-parallel over batch, 4 batches per core across 8 cores.
"""

import numpy as np

import concourse.bacc as bacc
import concourse.bass as bass
import concourse.tile as tile
from concourse import mybir
from concourse.bass_utils import run_bass_kernel_spmd

B, V, J, H, W = 32, 4, 17, 128, 128
N_CORES = 8
B_LOC = B // N_CORES          # 4 batches per core
GROUPS = B_LOC * V            # 16 (b,v) groups per core
SLICES = GROUPS * J           # 272 slices per core

_CACHE = {}


GPB = 2                    # (b,v) groups per block
NBLK = GROUPS // GPB       # blocks per core
JB = GPB * J               # joints (slices) per block

# chunk sizes (in slices) over the 272 per-core slices: small chunks at the
# start (fast pipeline ramp: compute starts after a ~1us DMA, not ~3us) and
# at the end (short tail after the last DMA lands)
CHUNKS = [4, 4, 4, 5] + [17] * 14 + [9, 8]
assert sum(CHUNKS) == SLICES


def _build_nc(passes=1, chunks=None, load_bufs=6):
    # Bacc (not raw Bass): its finalize() runs the legalization passes that
    # split multi-wait instructions (matmul can carry at most 1 sync wait).
    nc = bacc.Bacc()
    f32 = mybir.dt.float32
    chunks = list(CHUNKS) if chunks is None else list(chunks)
    nck = len(chunks)
    maxck = max(chunks)

    pred = nc.declare_dram_parameter("pred", [SLICES, H, W], f32, isOutput=False)
    gyt = nc.declare_dram_parameter("gyt", [H, SLICES], f32, isOutput=False)
    gxt = nc.declare_dram_parameter("gxt", [W, SLICES], f32, isOutput=False)
    partials = nc.declare_dram_parameter("partials", [128, 2, nck], f32, isOutput=True)

    with tile.TileContext(nc) as tc:
        with (
            tc.tile_pool(name="consts", bufs=1) as consts,
            tc.tile_pool(name="loads", bufs=load_bufs) as loads,
            tc.tile_pool(name="sq", bufs=2) as sqpool,
            tc.tile_pool(name="prod", bufs=2) as prodpool,
            tc.tile_pool(name="psum", bufs=4, space="PSUM") as psumpool,
            tc.tile_pool(name="outs", bufs=1) as outs,
        ):
            # warm-up ACT so the Square table-set load (~2.7us) overlaps the
            # first pred DMA instead of stalling the first real ACT
            warm = consts.tile([128, 1], f32)
            nc.vector.memset(warm[:], 0.0)
            wsq = consts.tile([128, 1], f32)
            nc.scalar.activation(
                out=wsq[:], in_=warm[:], func=mybir.ActivationFunctionType.Square
            )

            gyt_t = consts.tile([H, SLICES], f32)
            nc.sync.dma_start(out=gyt_t[:], in_=gyt[:, :])
            gxt_t = consts.tile([W, SLICES], f32)
            nc.sync.dma_start(out=gxt_t[:], in_=gxt[:, :])

            outcols = outs.tile([128, 2, nck], f32)

            for _p in range(passes):
                s0 = 0
                for c, csz in enumerate(chunks):
                    t = loads.tile([H, maxck, W], f32, tag="loads")
                    nc.sync.dma_start(
                        out=t[:, :csz, :],
                        in_=pred[s0 : s0 + csz].rearrange("s h w -> h s w"),
                    )

                    # s1: per-partition sum of pred^2 over (s, w)
                    sq = sqpool.tile([H, maxck, W], f32, tag="sq")
                    nc.scalar.activation(
                        out=sq[:, :csz, :],
                        in_=t[:, :csz, :],
                        func=mybir.ActivationFunctionType.Square,
                        accum_out=outcols[:, 0, c : c + 1],
                    )

                    # s2: m'_s = pred_s^T @ gy_s per slice -> psum column
                    ps = psumpool.tile([128, maxck], f32, tag="psum")
                    for sj in range(csz):
                        s = s0 + sj
                        nc.tensor.matmul(
                            ps[:, sj : sj + 1],
                            t[:, sj, :],
                            gyt_t[:, s : s + 1],
                            start=True,
                            stop=True,
                        )
                    # dot with gx, then per-partition sum over slices
                    prod = prodpool.tile([128, maxck], f32, tag="prod")
                    nc.vector.tensor_mul(
                        prod[:, :csz], ps[:, :csz], gxt_t[:, s0 : s0 + csz]
                    )
                    nc.vector.reduce_sum(
                        outcols[:, 1, c : c + 1], prod[:, :csz],
                        axis=mybir.AxisListType.X,
                    )
                    s0 += csz

            nc.sync.dma_start(out=partials[:, :, :], in_=outcols[:])

    nc.finalize()  # Bacc: runs legalization (wait splitting) + regalloc
    return nc


def _gaussians(proj_mats_batch, joints_3d_gt_batch):
    """1D gaussians gy [B,V,J,H], gx [B,V,J,W] in float32 (reference math)."""
    joints = joints_3d_gt_batch.astype(np.float32)
    ones = np.ones(joints.shape[:-1] + (1,), dtype=np.float32)
    joints_h = np.concatenate([joints, ones], axis=-1)  # [B, J, 4]
    proj = np.einsum(
        "bvcd,bjd->bvjc", proj_mats_batch.astype(np.float32), joints_h
    ).astype(np.float32)  # [B, V, J, 3]
    joints_2d = proj[..., :2] / proj[..., 2:3]  # (x, y)
    xs = np.arange(W, dtype=np.float32)
    ys = np.arange(H, dtype=np.float32)
    dx2 = (xs - joints_2d[..., 0, None]) ** 2  # [B,V,J,W]
    dy2 = (ys - joints_2d[..., 1, None]) ** 2  # [B,V,J,H]
    gx = np.exp(-0.5 * dx2).astype(np.float32)
    gy = np.exp(-0.5 * dy2).astype(np.float32)
    return gy, gx


def kernel(heatmaps_pred, proj_mats_batch, joints_3d_gt_batch, joints_3d_valid_batch,
           _profile=None):
    heatmaps_pred = np.ascontiguousarray(np.asarray(heatmaps_pred, dtype=np.float32))
    gy, gx = _gaussians(np.asarray(proj_mats_batch), np.asarray(joints_3d_gt_batch))

    # s3 = sum over slices of (sum_h gy^2) * (sum_w gx^2), exact in f64
    s3 = float(
        ((gy.astype(np.float64) ** 2).sum(-1) * (gx.astype(np.float64) ** 2).sum(-1)).sum()
    )

    if "nc" not in _CACHE:
        _CACHE["nc"] = _build_nc()
    nc = _CACHE["nc"]

    in_maps = []
    for c in range(N_CORES):
        bsl = slice(B_LOC * c, B_LOC * (c + 1))
        # slice order: (b_local, v, j) -> s ; tiles are [H|W, SLICES]
        gyt = np.ascontiguousarray(gy[bsl].reshape(SLICES, H).T)
        gxt = np.ascontiguousarray(gx[bsl].reshape(SLICES, W).T)
        in_maps.append(
            {
                "pred": heatmaps_pred[bsl].reshape(SLICES, H, W),
                "gyt": gyt,
                "gxt": gxt,
            }
        )

    res = run_bass_kernel_spmd(nc, in_maps, core_ids=list(range(N_CORES)))
    if _profile is not None:
        _profile["result"] = res
        _profile["in_maps"] = in_maps

    s1 = 0.0
    s2 = 0.0
    for c in range(N_CORES):
        p = res.results[c]["partials"].astype(np.float64)
        s1 += p[:, 0, :].sum()
        s2 += p[:, 1, :].sum()

    total = s1 - 2.0 * s2 + s3
    return np.float32(total / (B * V * J * H * W))
